# revision 6
# baseline (speedup 1.0000x reference)
"""BiLSTM-CRF loss kernel for 8 Trainium2 NeuronCores.

Math (per sequence):
  NLL = log Z - gold
  log Z:  forward algorithm over L=1024 steps, T=32 tags.
  gold:   score of the labelled path.

Device formulation (linear domain, periodically rescaled):
  a_{l+1} = diag(exp(f_l)) @ E^T @ a_l          E[j,i] = exp(trans[i,j])
  Z = sum_i a_L[i] * exp(trans[STOP, i])
  The gold score is the same recurrence with exp(f_l) masked to the
  labelled tag (one-hot), so it shares all device machinery.

Each core processes 128 sequences (pure batch data-parallel).  Four
independent chains ride the 128 SBUF partitions as 4 slices of 32 tags:
  slice 0: Z forward        slice 1: gold forward
  slice 2: Z backward       slice 3: gold backward
Forward chains cover steps 0..511, backward chains cover 1023..512 and
the two halves are joined with one extra matmul.  One 128x128
block-diagonal bf16 matmul + one DVE tensor-tensor multiply advance all
four chains by one step ("superstep"; 512 total).

Host-side staging only reorders/masks the inputs: feats are laid out as
[(slice, tag), superstep, seq] bf16, with the gold slices replaced by
feats-where-tag-matches / -inf elsewhere.  exp() happens on device.
"""

import sys

sys.path.insert(0, "/opt/trn_rl_repo")

import numpy as np
import ml_dtypes

B, L, T = 1024, 1024, 32
START, STOP = 30, 31
NCORES = 8
BS = B // NCORES          # sequences per core
S = L // 2                # supersteps
CH = 64                   # supersteps per DMA/exp/rescale chunk
NCH = S // CH
MASK_NEG = -60000.0       # exp(MASK_NEG + bias) == 0 in fp32/bf16
MU_Z = 3.88               # mean per-step log-growth of the Z chains
MU_G = 0.0                # mean per-step log-growth of the gold chains

_compiled = None


def _build_nc():
    import concourse.bacc as bacc
    import concourse.tile as tile
    import concourse.mybir as mybir
    import concourse.masks as masks
    from concourse.bass import AP, MemorySpace

    fp32 = mybir.dt.float32
    bf16 = mybir.dt.bfloat16

    nc = bacc.Bacc(
        "TRN2",
        target_bir_lowering=False,
        debug=False,
        enable_asserts=False,
        num_devices=NCORES,
    )
    staged_d = nc.dram_tensor("staged", [128, S * 128], bf16, kind="ExternalInput").ap()
    trans_d = nc.dram_tensor("trans", [T, T], fp32, kind="ExternalInput").ap()
    out_d = nc.dram_tensor("out", [BS, 1], fp32, kind="ExternalOutput").ap()

    from contextlib import ExitStack

    with tile.TileContext(nc) as tc, ExitStack() as ctx:
        singles = ctx.enter_context(tc.tile_pool(name="singles", bufs=1))
        st_pool = ctx.enter_context(tc.tile_pool(name="staged", bufs=2))
        fx_pool = ctx.enter_context(tc.tile_pool(name="fexp", bufs=2))
        rhs_pool = ctx.enter_context(tc.tile_pool(name="rhs", bufs=3))
        ps_pool = ctx.enter_context(tc.tile_pool(name="psum", bufs=2, space="PSUM"))
        psb_pool = ctx.enter_context(tc.tile_pool(name="psumb", bufs=1, space="PSUM"))
        sm_pool = ctx.enter_context(tc.tile_pool(name="small", bufs=2))

        # ---- constants -------------------------------------------------
        trans_rep = singles.tile([128, T], fp32, tag="trans_rep")
        for k in range(4):
            nc.sync.dma_start(out=trans_rep[32 * k : 32 * (k + 1), :], in_=trans_d)
        # E_rep[32k+i, j] = exp(trans[i, j])   (lhsT for the backward blocks)
        e_rep = singles.tile([128, T], bf16, tag="e_rep")
        nc.scalar.activation(e_rep[:], trans_rep[:], mybir.ActivationFunctionType.Exp)
        # E_repT[32k+j, i] = exp(trans[i, j])  (lhsT for the forward blocks)
        e_rept = singles.tile([128, T], bf16, tag="e_rept")
        nc.vector.transpose(e_rept[:], e_rep[:])

        # W1: block-diag stationary [(zf, gf) -> E^T-form, (zb, gb) -> E-form]
        w1 = singles.tile([128, 128], bf16, tag="w1")
        nc.vector.memset(w1[:], 0.0)
        nc.vector.tensor_copy(w1[0:32, 0:32], e_rept[0:32, :])
        nc.vector.tensor_copy(w1[32:64, 32:64], e_rept[32:64, :])
        nc.vector.tensor_copy(w1[64:96, 64:96], e_rep[64:96, :])
        nc.vector.tensor_copy(w1[96:128, 96:128], e_rep[96:128, :])

        # W2: final join; fwd state rows -> bwd-aligned output partitions
        w2 = singles.tile([128, 128], bf16, tag="w2")
        nc.vector.memset(w2[:], 0.0)
        nc.vector.tensor_copy(w2[0:32, 64:96], e_rept[0:32, :])
        nc.vector.tensor_copy(w2[32:64, 96:128], e_rept[32:64, :])

        ident = singles.tile([128, 128], bf16, tag="ident")
        masks.make_identity(nc, ident[:])

        # per-partition bias for the bulk exp: exp(feat - mu)
        bias = singles.tile([128, 1], fp32, tag="bias")
        nc.vector.memset(bias[0:32, :], -MU_Z)
        nc.vector.memset(bias[32:64, :], -MU_G)
        nc.vector.memset(bias[64:96, :], -MU_Z)
        nc.vector.memset(bias[96:128, :], -MU_G)

        # rescale log accumulator [seq, slice]
        acc = singles.tile([128, 4], fp32, tag="acc")
        nc.vector.memset(acc[:], 0.0)

        # ---- chunk 0 load + chain init --------------------------------
        def load_chunk(c):
            st = st_pool.tile([128, CH * 128], bf16, tag="st")
            nc.sync.dma_start(
                out=st[:], in_=staged_d[:, c * CH * 128 : (c + 1) * CH * 128]
            )
            fx = fx_pool.tile([128, CH * 128], bf16, tag="fx")
            nc.scalar.activation(
                fx[:], st[:], mybir.ActivationFunctionType.Exp, bias=bias[:]
            )
            return fx

        fx = load_chunk(0)

        rhs = rhs_pool.tile([128, 128], bf16, tag="rhs")
        nc.vector.memset(rhs[:], 0.0)
        # one-hot e_START rows for the two forward chains
        nc.gpsimd.affine_select(
            out=rhs[0:32, :], in_=rhs[0:32, :], pattern=[[0, 128]],
            compare_op=mybir.AluOpType.not_equal, fill=1.0,
            base=-START, channel_multiplier=1,
        )
        nc.gpsimd.affine_select(
            out=rhs[32:64, :], in_=rhs[32:64, :], pattern=[[0, 128]],
            compare_op=mybir.AluOpType.not_equal, fill=1.0,
            base=-START, channel_multiplier=1,
        )
        # backward init: c_1023 = fexp_1023 * expstop   (slot 0 of bwd slices)
        expstop = singles.tile([128, 1], fp32, tag="expstop")
        nc.vector.tensor_copy(expstop[:], e_rept[:, STOP : STOP + 1])
        nc.scalar.mul(rhs[64:128, :], fx[64:128, 0:128], expstop[64:128, :])

        # ---- main loop -------------------------------------------------
        def rescale(state):
            pst = psb_pool.tile([128, 128], bf16, tag="pst")
            nc.tensor.transpose(pst[:], state[:], ident[:])
            pst3 = pst[:].rearrange("p (s t) -> p s t", t=32)
            mx = sm_pool.tile([128, 4], fp32, tag="mx")
            nc.vector.tensor_reduce(
                mx[:], pst3, axis=mybir.AxisListType.X, op=mybir.AluOpType.max
            )
            lg = sm_pool.tile([128, 4], fp32, tag="lg")
            nc.scalar.activation(lg[:], mx[:], mybir.ActivationFunctionType.Ln)
            nc.vector.tensor_add(acc[:], acc[:], lg[:])
            rcp = sm_pool.tile([128, 4], fp32, tag="rcp")
            nc.vector.reciprocal(rcp[:], mx[:])
            rcp_b = AP(
                tensor=rcp[:].tensor,
                offset=rcp[:].offset,
                ap=[rcp[:].ap[0], rcp[:].ap[1], [0, 32]],
            )
            st2 = sm_pool.tile([128, 128], bf16, tag="st2")
            nc.vector.tensor_mul(st2[:].rearrange("p (s t) -> p s t", t=32), pst3, rcp_b)
            psb = psb_pool.tile([128, 128], bf16, tag="psb")
            nc.tensor.transpose(psb[:], st2[:], ident[:])
            out = rhs_pool.tile([128, 128], bf16, tag="rhs")
            nc.vector.tensor_copy(out[:], psb[:])
            return out

        for s in range(S):
            c, sl = divmod(s, CH)
            if sl == 0 and s > 0:
                fx = load_chunk(c)
            ps = ps_pool.tile([128, 128], fp32, tag="ps")
            nc.tensor.matmul(ps[:], w1[:], rhs[:], start=True, stop=True)
            nrhs = rhs_pool.tile([128, 128], bf16, tag="rhs")
            fsl = fx[:, sl * 128 : (sl + 1) * 128]
            if s == 0:
                nc.vector.tensor_mul(nrhs[0:64, :], ps[0:64, :], fsl[0:64, :])
                nc.vector.tensor_copy(nrhs[64:128, :], rhs[64:128, :])
            else:
                nc.vector.tensor_mul(nrhs[:], ps[:], fsl)
            rhs = nrhs
            if sl == CH - 1:
                rhs = rescale(rhs)

        # ---- final join ------------------------------------------------
        psf = ps_pool.tile([128, 128], fp32, tag="ps")
        nc.tensor.matmul(psf[:], w2[:], rhs[:], start=True, stop=True)
        prod = sm_pool.tile([64, 128], bf16, tag="prod")
        nc.vector.tensor_mul(prod[:], psf[64:128, :], rhs[64:128, :])
        pst = psb_pool.tile([128, 64], bf16, tag="pst2")
        nc.tensor.transpose(pst[:], prod[:], ident[0:64, 0:64])
        zg = sm_pool.tile([128, 2], fp32, tag="zg")
        nc.vector.tensor_reduce(
            zg[:],
            pst[:].rearrange("p (s t) -> p s t", t=32),
            axis=mybir.AxisListType.X,
            op=mybir.AluOpType.add,
        )
        lzg = sm_pool.tile([128, 2], fp32, tag="lzg")
        nc.scalar.activation(lzg[:], zg[:], mybir.ActivationFunctionType.Ln)
        # nll = (lz - lg) + (acc0 + acc2 - acc1 - acc3) + L * (MU_Z - MU_G)
        t0 = sm_pool.tile([128, 1], fp32, tag="t0")
        nc.vector.tensor_sub(t0[:], lzg[:, 0:1], lzg[:, 1:2])
        t1 = sm_pool.tile([128, 1], fp32, tag="t1")
        nc.vector.tensor_add(t1[:], acc[:, 0:1], acc[:, 2:3])
        t2 = sm_pool.tile([128, 1], fp32, tag="t2")
        nc.vector.tensor_add(t2[:], acc[:, 1:2], acc[:, 3:4])
        t3 = sm_pool.tile([128, 1], fp32, tag="t3")
        nc.vector.tensor_sub(t3[:], t1[:], t2[:])
        res = sm_pool.tile([128, 1], fp32, tag="res")
        nc.vector.tensor_add(res[:], t0[:], t3[:])
        nc.vector.tensor_scalar_add(res[:], res[:], float(L) * (MU_Z - MU_G))
        nc.sync.dma_start(out=out_d, in_=res[:])

    nc.compile()
    return nc


def _stage_core(feats_c, tags_c):
    """feats_c [128, 1024, 32] f32, tags_c [128, 1024] int -> [128, S*128] bf16."""
    ft = np.ascontiguousarray(feats_c.transpose(2, 1, 0))        # [t, l, b]
    mask = tags_c[None, :, :] == np.arange(T, dtype=tags_c.dtype)[:, None, None]
    # mask[t, b, l] -> want [t, l, b]
    mask = mask.transpose(0, 2, 1)
    gt = np.where(mask, ft, np.float32(MASK_NEG))
    staged = np.empty((4, T, S, BS), np.float32)
    staged[0] = ft[:, :S, :]
    staged[1] = gt[:, :S, :]
    staged[2] = ft[:, ::-1, :][:, :S, :]
    staged[3] = gt[:, ::-1, :][:, :S, :]
    return staged.reshape(128, S * BS).astype(ml_dtypes.bfloat16)


LAST_RESULTS = None


def kernel(feats, transitions, tags, _trace=False):
    global _compiled, LAST_RESULTS
    from concourse.bass_utils import run_bass_kernel_spmd

    feats = np.asarray(feats, dtype=np.float32)
    transitions = np.asarray(transitions, dtype=np.float32)
    tags = np.asarray(tags)

    if _compiled is None:
        _compiled = _build_nc()
    nc = _compiled

    in_maps = []
    for c in range(NCORES):
        sl = slice(c * BS, (c + 1) * BS)
        in_maps.append(
            {
                "staged": _stage_core(feats[sl], tags[sl]),
                "trans": transitions,
            }
        )
    res = run_bass_kernel_spmd(
        nc, in_maps, core_ids=list(range(NCORES)), trace=_trace
    )
    LAST_RESULTS = res
    out = np.concatenate([r["out"].reshape(BS) for r in res.results])
    return out.astype(np.float32)


# revision 15
# speedup vs baseline: 5269.1503x; 5269.1503x over previous
"""BiLSTM-CRF loss kernel for 8 Trainium2 NeuronCores.

Math (per sequence):
  NLL = log Z - gold
  log Z:  forward algorithm over L=1024 steps, T=32 tags.
  gold:   score of the labelled path.

Device formulation (linear domain, periodically rescaled):
  a_{l+1} = diag(exp(f_l)) @ E^T @ a_l          E[j,i] = exp(trans[i,j])
  Z = sum_i a_L[i] * exp(trans[STOP, i])
  The gold score is the same recurrence with exp(f_l) masked to the
  labelled tag (one-hot), so it shares all device machinery.

Each core processes 128 sequences (pure batch data-parallel).  Four
independent chains ride the 128 SBUF partitions as 4 slices of 32 tags:
  slice 0: Z forward        slice 1: gold forward
  slice 2: Z backward       slice 3: gold backward
Forward chains cover steps 0..511, backward chains cover 1023..512 and
the halves are joined with one extra matmul.  One 128x128
block-diagonal bf16 matmul + one DVE tensor-tensor multiply advance all
four chains by one step.  To hide the PE->PSUM->DVE latency the 128
sequences are further split into two independent half-chains (64 seqs
each) that software-pipeline against each other; 512 supersteps total.

Host-side staging only reorders/masks the inputs: feats are laid out as
[(slice, tag), superstep, seq] bf16, with the gold slices replaced by
feats-where-tag-matches / -inf elsewhere.  exp() happens on device.
"""

import sys

sys.path.insert(0, "/opt/trn_rl_repo")

import numpy as np
import ml_dtypes

B, L, T = 1024, 1024, 32
START, STOP = 30, 31
NCORES = 8
BS = B // NCORES          # sequences per core
HB = BS // 2              # sequences per half-chain
S = L // 2                # supersteps
CH = 64                   # supersteps per DMA/exp chunk
NCH = S // CH
RESCALE_EVERY = 128       # supersteps between rescales
MASK_NEG = -60000.0       # exp(MASK_NEG + bias) == 0 in fp32/bf16
MU_Z = 3.88               # mean per-step log-growth of the Z chains
MU_G = 0.0                # mean per-step log-growth of the gold chains

_compiled = None


def _build_nc():
    import concourse.bacc as bacc
    import concourse.tile as tile
    import concourse.mybir as mybir
    import concourse.masks as masks
    from concourse.bass import AP

    fp32 = mybir.dt.float32
    bf16 = mybir.dt.bfloat16

    nc = bacc.Bacc(
        "TRN2",
        target_bir_lowering=False,
        debug=False,
        enable_asserts=False,
        num_devices=NCORES,
    )
    staged_d = nc.dram_tensor("staged", [128, S * 128], bf16, kind="ExternalInput").ap()
    trans_d = nc.dram_tensor("trans", [T, T], fp32, kind="ExternalInput").ap()
    out_d = nc.dram_tensor("out", [BS, 1], fp32, kind="ExternalOutput").ap()

    from contextlib import ExitStack

    with tile.TileContext(nc) as tc, ExitStack() as ctx:
        singles = ctx.enter_context(tc.tile_pool(name="singles", bufs=1))
        st_pool = ctx.enter_context(tc.tile_pool(name="staged", bufs=2))
        fx_pool = ctx.enter_context(tc.tile_pool(name="fexp", bufs=2))
        rhs_pool = ctx.enter_context(tc.tile_pool(name="rhs", bufs=4))
        ps_pool = ctx.enter_context(tc.tile_pool(name="psum", bufs=2, space="PSUM"))
        psb_pool = ctx.enter_context(tc.tile_pool(name="psumb", bufs=1, space="PSUM"))
        sm_pool = ctx.enter_context(tc.tile_pool(name="small", bufs=2))

        # ---- constants -------------------------------------------------
        trans_rep = singles.tile([128, T], fp32, tag="trans_rep")
        for k in range(4):
            nc.sync.dma_start(out=trans_rep[32 * k : 32 * (k + 1), :], in_=trans_d)
        # E_rep[32k+i, j] = exp(trans[i, j])   (lhsT for the backward blocks)
        e_rep = singles.tile([128, T], bf16, tag="e_rep")
        nc.scalar.activation(e_rep[:], trans_rep[:], mybir.ActivationFunctionType.Exp)
        # E_repT[32k+j, i] = exp(trans[i, j])  (lhsT for the forward blocks)
        e_rept = singles.tile([128, T], bf16, tag="e_rept")
        nc.vector.transpose(e_rept[:], e_rep[:])

        # W1: block-diag stationary [(zf, gf) -> E^T-form, (zb, gb) -> E-form]
        w1 = singles.tile([128, 128], bf16, tag="w1")
        nc.vector.memset(w1[:], 0.0)
        nc.vector.tensor_copy(w1[0:32, 0:32], e_rept[0:32, :])
        nc.vector.tensor_copy(w1[32:64, 32:64], e_rept[32:64, :])
        nc.vector.tensor_copy(w1[64:96, 64:96], e_rep[64:96, :])
        nc.vector.tensor_copy(w1[96:128, 96:128], e_rep[96:128, :])

        # W2: final join; fwd state rows -> bwd-aligned output partitions
        w2 = singles.tile([128, 128], bf16, tag="w2")
        nc.vector.memset(w2[:], 0.0)
        nc.vector.tensor_copy(w2[0:32, 64:96], e_rept[0:32, :])
        nc.vector.tensor_copy(w2[32:64, 96:128], e_rept[32:64, :])

        ident = singles.tile([128, 128], bf16, tag="ident")
        masks.make_identity(nc, ident[:])

        # per-partition bias for the bulk exp: exp(feat - mu)
        bias = singles.tile([128, 1], fp32, tag="bias")
        nc.vector.memset(bias[0:32, :], -MU_Z)
        nc.vector.memset(bias[32:64, :], -MU_G)
        nc.vector.memset(bias[64:96, :], -MU_Z)
        nc.vector.memset(bias[96:128, :], -MU_G)

        # rescale log accumulators [seq-in-half, slice], one per half-chain
        accs = []
        for h in range(2):
            a = singles.tile([HB, 4], fp32, tag=f"acc{h}")
            nc.vector.memset(a[:], 0.0)
            accs.append(a)

        # ---- chunk loading --------------------------------------------
        def load_chunk(c):
            st = st_pool.tile([128, CH * 128], bf16, tag="st")
            nc.sync.dma_start(
                out=st[:], in_=staged_d[:, c * CH * 128 : (c + 1) * CH * 128]
            )
            fx = fx_pool.tile([128, CH * 128], bf16, tag="fx")
            nc.scalar.activation(
                fx[:], st[:], mybir.ActivationFunctionType.Exp, bias=bias[:]
            )
            return fx

        fx = load_chunk(0)

        # ---- chain init ------------------------------------------------
        expstop = singles.tile([128, 1], fp32, tag="expstop")
        nc.vector.tensor_copy(expstop[:], e_rept[:, STOP : STOP + 1])

        rhs = []
        for h in range(2):
            r = rhs_pool.tile([128, HB], bf16, tag=f"rhs{h}")
            nc.vector.memset(r[:], 0.0)
            for sl in (0, 32):
                nc.gpsimd.affine_select(
                    out=r[sl : sl + 32, :], in_=r[sl : sl + 32, :],
                    pattern=[[0, HB]],
                    compare_op=mybir.AluOpType.not_equal, fill=1.0,
                    base=-START, channel_multiplier=1,
                )
            # backward init: c_1023 = fexp_1023 * expstop  (slot 0, this half)
            nc.scalar.mul(
                r[64:128, :], fx[64:128, h * HB : h * HB + HB], expstop[64:128, :]
            )
            rhs.append(r)

        # ---- rescale ---------------------------------------------------
        def rescale(h, state):
            pst = psb_pool.tile([HB, 128], bf16, tag="pst")
            nc.tensor.matmul(pst[:], state[:], ident[:, 0:128], is_transpose=True)
            pst3 = pst[:].rearrange("p (s t) -> p s t", t=32)
            mx = sm_pool.tile([HB, 4], fp32, tag="mx")
            nc.vector.tensor_reduce(
                mx[:], pst3, axis=mybir.AxisListType.X, op=mybir.AluOpType.max
            )
            lg = sm_pool.tile([HB, 4], fp32, tag="lg")
            nc.scalar.activation(lg[:], mx[:], mybir.ActivationFunctionType.Ln)
            nc.vector.tensor_add(accs[h][:], accs[h][:], lg[:])
            rcp = sm_pool.tile([HB, 4], fp32, tag="rcp")
            nc.vector.reciprocal(rcp[:], mx[:])
            rcp_b = AP(
                tensor=rcp[:].tensor,
                offset=rcp[:].offset,
                ap=[rcp[:].ap[0], rcp[:].ap[1], [0, 32]],
            )
            st2 = sm_pool.tile([HB, 128], bf16, tag="st2")
            nc.vector.tensor_mul(
                st2[:].rearrange("p (s t) -> p s t", t=32), pst3, rcp_b
            )
            psb = psb_pool.tile([128, HB], bf16, tag="psb")
            nc.tensor.matmul(psb[:], st2[:], ident[0:HB, 0:HB], is_transpose=True)
            out = rhs_pool.tile([128, HB], bf16, tag=f"rhs{h}")
            nc.vector.tensor_copy(out[:], psb[:])
            return out

        # ---- main loop -------------------------------------------------
        for s in range(S):
            c, sl = divmod(s, CH)
            if sl == 0 and s > 0:
                fx = load_chunk(c)
            ps = [None, None]
            for h in range(2):
                ps[h] = ps_pool.tile([128, HB], fp32, tag=f"ps{h}", name=f"ps{h}_{s}")
                nc.tensor.matmul(ps[h][:], w1[:], rhs[h][:], start=True, stop=True)
            for h in range(2):
                nrhs = rhs_pool.tile([128, HB], bf16, tag=f"rhs{h}")
                fsl = fx[:, sl * 128 + h * HB : sl * 128 + h * HB + HB]
                if s == 0:
                    nc.vector.tensor_mul(nrhs[0:64, :], ps[h][0:64, :], fsl[0:64, :])
                    nc.vector.tensor_copy(nrhs[64:128, :], rhs[h][64:128, :])
                else:
                    nc.vector.tensor_mul(nrhs[:], ps[h][:], fsl)
                rhs[h] = nrhs
            if s % RESCALE_EVERY == RESCALE_EVERY - 1:
                for h in range(2):
                    rhs[h] = rescale(h, rhs[h])

        # ---- final join ------------------------------------------------
        for h in range(2):
            psf = ps_pool.tile([128, HB], fp32, tag=f"ps{h}")
            nc.tensor.matmul(psf[:], w2[:], rhs[h][:], start=True, stop=True)
            # TT operands must share partitions; psf/rhs slices are on 64:128,
            # so allocate a [128, HB] tile and use its upper half.
            prod128 = sm_pool.tile([128, HB], bf16, tag="prod128")
            nc.vector.tensor_mul(
                prod128[64:128, :], psf[64:128, :], rhs[h][64:128, :]
            )
            pst = psb_pool.tile([HB, 64], bf16, tag="pst2")
            nc.tensor.matmul(
                pst[:], prod128[64:128, :], ident[64:128, 64:128],
                is_transpose=True, tile_position=(64, 0),
            )
            zg = sm_pool.tile([HB, 2], fp32, tag="zg")
            nc.vector.tensor_reduce(
                zg[:],
                pst[:].rearrange("p (s t) -> p s t", t=32),
                axis=mybir.AxisListType.X,
                op=mybir.AluOpType.add,
            )
            lzg = sm_pool.tile([HB, 2], fp32, tag="lzg")
            nc.scalar.activation(lzg[:], zg[:], mybir.ActivationFunctionType.Ln)
            # nll = (lz - lg) + (acc0 + acc2 - acc1 - acc3) + L * (MU_Z - MU_G)
            t0 = sm_pool.tile([HB, 1], fp32, tag="t0")
            nc.vector.tensor_sub(t0[:], lzg[:, 0:1], lzg[:, 1:2])
            t1 = sm_pool.tile([HB, 1], fp32, tag="t1")
            nc.vector.tensor_add(t1[:], accs[h][:, 0:1], accs[h][:, 2:3])
            t2 = sm_pool.tile([HB, 1], fp32, tag="t2")
            nc.vector.tensor_add(t2[:], accs[h][:, 1:2], accs[h][:, 3:4])
            t3 = sm_pool.tile([HB, 1], fp32, tag="t3")
            nc.vector.tensor_sub(t3[:], t1[:], t2[:])
            t4 = sm_pool.tile([HB, 1], fp32, tag="t4")
            nc.vector.tensor_add(t4[:], t0[:], t3[:])
            res_h = sm_pool.tile([HB, 1], fp32, tag=f"res{h}")
            nc.vector.tensor_scalar_add(res_h[:], t4[:], float(L) * (MU_Z - MU_G))
            nc.sync.dma_start(out=out_d[h * HB : (h + 1) * HB, :], in_=res_h[:])

    nc.compile()
    return nc


def _stage_core(feats_c, tags_c):
    """feats_c [128, 1024, 32] f32, tags_c [128, 1024] int -> [128, S*128] bf16."""
    ft = np.ascontiguousarray(feats_c.transpose(2, 1, 0))        # [t, l, b]
    mask = tags_c[None, :, :] == np.arange(T, dtype=tags_c.dtype)[:, None, None]
    # mask[t, b, l] -> want [t, l, b]
    mask = mask.transpose(0, 2, 1)
    gt = np.where(mask, ft, np.float32(MASK_NEG))
    staged = np.empty((4, T, S, BS), np.float32)
    staged[0] = ft[:, :S, :]
    staged[1] = gt[:, :S, :]
    staged[2] = ft[:, ::-1, :][:, :S, :]
    staged[3] = gt[:, ::-1, :][:, :S, :]
    return staged.reshape(128, S * BS).astype(ml_dtypes.bfloat16)


LAST_RESULTS = None


def kernel(feats, transitions, tags, _trace=False):
    global _compiled, LAST_RESULTS
    from concourse.bass_utils import run_bass_kernel_spmd

    feats = np.asarray(feats, dtype=np.float32)
    transitions = np.asarray(transitions, dtype=np.float32)
    tags = np.asarray(tags)

    if _compiled is None:
        _compiled = _build_nc()
    nc = _compiled

    in_maps = []
    for c in range(NCORES):
        sl = slice(c * BS, (c + 1) * BS)
        in_maps.append(
            {
                "staged": _stage_core(feats[sl], tags[sl]),
                "trans": transitions,
            }
        )
    res = run_bass_kernel_spmd(
        nc, in_maps, core_ids=list(range(NCORES)), trace=_trace
    )
    LAST_RESULTS = res
    out = np.concatenate([r["out"].reshape(BS) for r in res.results])
    return out.astype(np.float32)


# revision 20
# speedup vs baseline: 5508.8769x; 1.0455x over previous
"""BiLSTM-CRF loss kernel for 8 Trainium2 NeuronCores.

Math (per sequence):
  NLL = log Z - gold
  log Z:  forward algorithm over L=1024 steps, T=32 tags.
  gold:   score of the labelled path.

Device formulation (linear domain, periodically rescaled):
  a_{l+1} = diag(exp(f_l)) @ E^T @ a_l          E[j,i] = exp(trans[i,j])
  Z = sum_i a_L[i] * exp(trans[STOP, i])
  The gold score is the same recurrence with exp(f_l) masked to the
  labelled tag (one-hot), so it shares all device machinery.

Each core processes 128 sequences (pure batch data-parallel).  Four
independent chains ride the 128 SBUF partitions as 4 slices of 32 tags:
  slice 0: Z forward        slice 1: gold forward
  slice 2: Z backward       slice 3: gold backward
Forward chains cover steps 0..511, backward chains cover 1023..512 and
the halves are joined with one extra matmul.  One 128x128
block-diagonal bf16 matmul + one DVE tensor-tensor multiply advance all
four chains by one step.  To hide the PE->PSUM->DVE latency the 128
sequences are further split into two independent half-chains (64 seqs
each) that software-pipeline against each other; 512 supersteps total.

Host-side staging only reorders/masks the inputs: feats are laid out as
[(slice, tag), superstep, seq] bf16, with the gold slices replaced by
feats-where-tag-matches / -inf elsewhere.  exp() happens on device.
"""

import sys

sys.path.insert(0, "/opt/trn_rl_repo")

import numpy as np
import ml_dtypes

B, L, T = 1024, 1024, 32
START, STOP = 30, 31
NCORES = 8
BS = B // NCORES          # sequences per core
HB = BS // 2              # sequences per half-chain (legacy name)
GROUPS = [(0, 64), (64, 64)]             # (seq offset, size) per chain group
S = L // 2                # supersteps
CH = 64                   # supersteps per DMA/exp chunk
NCH = S // CH
RESCALE_EVERY = 128       # supersteps between rescales
MASK_NEG = -60000.0       # exp(MASK_NEG + bias) == 0 in fp32/bf16
MU_Z = 3.88               # mean per-step log-growth of the Z chains
MU_G = 0.0                # mean per-step log-growth of the gold chains

_compiled = None


def _build_nc():
    import concourse.bacc as bacc
    import concourse.tile as tile
    import concourse.mybir as mybir
    import concourse.masks as masks
    from concourse.bass import AP

    fp32 = mybir.dt.float32
    bf16 = mybir.dt.bfloat16

    nc = bacc.Bacc(
        "TRN2",
        target_bir_lowering=False,
        debug=False,
        enable_asserts=False,
        num_devices=NCORES,
    )
    staged_d = nc.dram_tensor("staged", [128, S * 128], bf16, kind="ExternalInput").ap()
    trans_d = nc.dram_tensor("trans", [T, T], fp32, kind="ExternalInput").ap()
    out_d = nc.dram_tensor("out", [BS, 1], fp32, kind="ExternalOutput").ap()

    from contextlib import ExitStack

    with tile.TileContext(nc) as tc, ExitStack() as ctx:
        singles = ctx.enter_context(tc.tile_pool(name="singles", bufs=1))
        st_pool = ctx.enter_context(tc.tile_pool(name="staged", bufs=2))
        fx_pool = ctx.enter_context(tc.tile_pool(name="fexp", bufs=2))
        rhs_pool = ctx.enter_context(tc.tile_pool(name="rhs", bufs=4))
        ps_pool = ctx.enter_context(tc.tile_pool(name="psum", bufs=2, space="PSUM"))
        psb_pool = ctx.enter_context(tc.tile_pool(name="psumb", bufs=1, space="PSUM"))
        sm_pool = ctx.enter_context(tc.tile_pool(name="small", bufs=2))

        # ---- constants -------------------------------------------------
        trans_rep = singles.tile([128, T], fp32, tag="trans_rep")
        for k in range(4):
            nc.sync.dma_start(out=trans_rep[32 * k : 32 * (k + 1), :], in_=trans_d)
        # E_rep[32k+i, j] = exp(trans[i, j])   (lhsT for the backward blocks)
        e_rep = singles.tile([128, T], bf16, tag="e_rep")
        nc.scalar.activation(e_rep[:], trans_rep[:], mybir.ActivationFunctionType.Exp)
        # E_repT[32k+j, i] = exp(trans[i, j])  (lhsT for the forward blocks)
        e_rept = singles.tile([128, T], bf16, tag="e_rept")
        nc.vector.transpose(e_rept[:], e_rep[:])

        # W1: block-diag stationary [(zf, gf) -> E^T-form, (zb, gb) -> E-form]
        w1 = singles.tile([128, 128], bf16, tag="w1")
        nc.vector.memset(w1[:], 0.0)
        nc.vector.tensor_copy(w1[0:32, 0:32], e_rept[0:32, :])
        nc.vector.tensor_copy(w1[32:64, 32:64], e_rept[32:64, :])
        nc.vector.tensor_copy(w1[64:96, 64:96], e_rep[64:96, :])
        nc.vector.tensor_copy(w1[96:128, 96:128], e_rep[96:128, :])

        # W2: final join; fwd state rows -> bwd-aligned output partitions
        w2 = singles.tile([128, 128], bf16, tag="w2")
        nc.vector.memset(w2[:], 0.0)
        nc.vector.tensor_copy(w2[0:32, 64:96], e_rept[0:32, :])
        nc.vector.tensor_copy(w2[32:64, 96:128], e_rept[32:64, :])

        ident = singles.tile([128, 128], bf16, tag="ident")
        masks.make_identity(nc, ident[:])

        # per-partition bias for the bulk exp: exp(feat - mu)
        bias = singles.tile([128, 1], fp32, tag="bias")
        nc.vector.memset(bias[0:32, :], -MU_Z)
        nc.vector.memset(bias[32:64, :], -MU_G)
        nc.vector.memset(bias[64:96, :], -MU_Z)
        nc.vector.memset(bias[96:128, :], -MU_G)

        # rescale log accumulators [seq-in-group, slice], one per chain group
        accs = []
        for h, (off, gsz) in enumerate(GROUPS):
            a = singles.tile([gsz, 4], fp32, tag=f"acc{h}")
            nc.vector.memset(a[:], 0.0)
            accs.append(a)

        # ---- chunk loading --------------------------------------------
        # small leading chunks so the chains start early; steady-state CH
        chunk_sched = [(0, 4), (4, 12), (16, 48)]
        while chunk_sched[-1][0] + chunk_sched[-1][1] < S:
            c0 = chunk_sched[-1][0] + chunk_sched[-1][1]
            chunk_sched.append((c0, min(CH, S - c0)))

        def load_chunk(c0, clen):
            st = st_pool.tile([128, clen * 128], bf16, tag="st", name=f"st_{c0}")
            nc.sync.dma_start(
                out=st[:], in_=staged_d[:, c0 * 128 : (c0 + clen) * 128]
            )
            fx = fx_pool.tile([128, clen * 128], bf16, tag="fx", name=f"fx_{c0}")
            nc.scalar.activation(
                fx[:], st[:], mybir.ActivationFunctionType.Exp, bias=bias[:]
            )
            return fx

        fx = load_chunk(*chunk_sched[0])

        # ---- chain init ------------------------------------------------
        expstop = singles.tile([128, 1], fp32, tag="expstop")
        nc.vector.tensor_copy(expstop[:], e_rept[:, STOP : STOP + 1])

        rhs = []
        for h, (off, gsz) in enumerate(GROUPS):
            r = rhs_pool.tile([128, gsz], bf16, tag=f"rhs{h}", name=f"rhs{h}_i")
            nc.vector.memset(r[:], 0.0)
            for sl in (0, 32):
                nc.gpsimd.affine_select(
                    out=r[sl : sl + 32, :], in_=r[sl : sl + 32, :],
                    pattern=[[0, gsz]],
                    compare_op=mybir.AluOpType.not_equal, fill=1.0,
                    base=-START, channel_multiplier=1,
                )
            # backward init: c_1023 = fexp_1023 * expstop  (slot 0, this group)
            nc.scalar.mul(
                r[64:128, :], fx[64:128, off : off + gsz], expstop[64:128, :]
            )
            rhs.append(r)

        # ---- rescale ---------------------------------------------------
        def rescale(h, state, s):
            gsz = GROUPS[h][1]
            pst = psb_pool.tile([gsz, 128], bf16, tag="psx", name=f"pst{h}_{s}")
            nc.tensor.matmul(pst[:], state[:], ident[:, 0:128], is_transpose=True)
            pst3 = pst[:].rearrange("p (s t) -> p s t", t=32)
            mx = sm_pool.tile([gsz, 4], fp32, tag="mx")
            nc.vector.tensor_reduce(
                mx[:], pst3, axis=mybir.AxisListType.X, op=mybir.AluOpType.max
            )
            lg = sm_pool.tile([gsz, 4], fp32, tag="lg")
            nc.scalar.activation(lg[:], mx[:], mybir.ActivationFunctionType.Ln)
            nc.vector.tensor_add(accs[h][:], accs[h][:], lg[:])
            rcp = sm_pool.tile([gsz, 4], fp32, tag="rcp")
            nc.vector.reciprocal(rcp[:], mx[:])
            rcp_b = AP(
                tensor=rcp[:].tensor,
                offset=rcp[:].offset,
                ap=[rcp[:].ap[0], rcp[:].ap[1], [0, 32]],
            )
            st2 = sm_pool.tile([gsz, 128], bf16, tag="st2")
            nc.vector.tensor_mul(
                st2[:].rearrange("p (s t) -> p s t", t=32), pst3, rcp_b
            )
            psb = psb_pool.tile([128, gsz], bf16, tag="psx", name=f"psb{h}_{s}")
            nc.tensor.matmul(psb[:], st2[:], ident[0:gsz, 0:gsz], is_transpose=True)
            out = rhs_pool.tile([128, gsz], bf16, tag=f"rhs{h}", name=f"rhsr{h}_{s}")
            nc.vector.tensor_copy(out[:], psb[:])
            return out

        # ---- main loop -------------------------------------------------
        chunk_idx = 0
        for s in range(S):
            if s >= chunk_sched[chunk_idx][0] + chunk_sched[chunk_idx][1]:
                chunk_idx += 1
                fx = load_chunk(*chunk_sched[chunk_idx])
            sl = s - chunk_sched[chunk_idx][0]
            ps = [None] * len(GROUPS)
            for h, (off, gsz) in enumerate(GROUPS):
                ps[h] = ps_pool.tile([128, gsz], fp32, tag=f"ps{h}", name=f"ps{h}_{s}")
                nc.tensor.matmul(ps[h][:], w1[:], rhs[h][:], start=True, stop=True)
            for h, (off, gsz) in enumerate(GROUPS):
                nrhs = rhs_pool.tile([128, gsz], bf16, tag=f"rhs{h}", name=f"rhs{h}_{s}")
                fsl = fx[:, sl * 128 + off : sl * 128 + off + gsz]
                if s == 0:
                    nc.vector.tensor_mul(nrhs[0:64, :], ps[h][0:64, :], fsl[0:64, :])
                    nc.vector.tensor_copy(nrhs[64:128, :], rhs[h][64:128, :])
                else:
                    nc.vector.tensor_mul(nrhs[:], ps[h][:], fsl)
                rhs[h] = nrhs
            if s % RESCALE_EVERY == RESCALE_EVERY - 1:
                for h in range(len(GROUPS)):
                    rhs[h] = rescale(h, rhs[h], s)

        # ---- final join ------------------------------------------------
        for h, (off, gsz) in enumerate(GROUPS):
            psf = ps_pool.tile([128, gsz], fp32, tag=f"ps{h}", name=f"psf{h}")
            nc.tensor.matmul(psf[:], w2[:], rhs[h][:], start=True, stop=True)
            # TT operands must share partitions; psf/rhs slices are on 64:128,
            # so allocate a [128, gsz] tile and use its upper half.
            prod128 = sm_pool.tile([128, gsz], bf16, tag="prod128", name=f"prod{h}")
            nc.vector.tensor_mul(
                prod128[64:128, :], psf[64:128, :], rhs[h][64:128, :]
            )
            pst = psb_pool.tile([gsz, 64], bf16, tag="psx", name=f"pstf{h}")
            nc.tensor.matmul(
                pst[:], prod128[64:128, :], ident[64:128, 64:128],
                is_transpose=True, tile_position=(64, 0),
            )
            zg = sm_pool.tile([gsz, 2], fp32, tag="zg")
            nc.vector.tensor_reduce(
                zg[:],
                pst[:].rearrange("p (s t) -> p s t", t=32),
                axis=mybir.AxisListType.X,
                op=mybir.AluOpType.add,
            )
            lzg = sm_pool.tile([gsz, 2], fp32, tag="lzg")
            nc.scalar.activation(lzg[:], zg[:], mybir.ActivationFunctionType.Ln)
            # nll = (lz - lg) + (acc0 + acc2 - acc1 - acc3) + L * (MU_Z - MU_G)
            t0 = sm_pool.tile([gsz, 1], fp32, tag="t0")
            nc.vector.tensor_sub(t0[:], lzg[:, 0:1], lzg[:, 1:2])
            t1 = sm_pool.tile([gsz, 1], fp32, tag="t1")
            nc.vector.tensor_add(t1[:], accs[h][:, 0:1], accs[h][:, 2:3])
            t2 = sm_pool.tile([gsz, 1], fp32, tag="t2")
            nc.vector.tensor_add(t2[:], accs[h][:, 1:2], accs[h][:, 3:4])
            t3 = sm_pool.tile([gsz, 1], fp32, tag="t3")
            nc.vector.tensor_sub(t3[:], t1[:], t2[:])
            t4 = sm_pool.tile([gsz, 1], fp32, tag="t4")
            nc.vector.tensor_add(t4[:], t0[:], t3[:])
            res_h = sm_pool.tile([gsz, 1], fp32, tag=f"res{h}")
            nc.vector.tensor_scalar_add(res_h[:], t4[:], float(L) * (MU_Z - MU_G))
            nc.sync.dma_start(out=out_d[off : off + gsz, :], in_=res_h[:])

    nc.compile()
    return nc


def _stage_core(feats_c, tags_c):
    """feats_c [128, 1024, 32] f32, tags_c [128, 1024] int -> [128, S*128] bf16."""
    ft = np.ascontiguousarray(feats_c.transpose(2, 1, 0))        # [t, l, b]
    mask = tags_c[None, :, :] == np.arange(T, dtype=tags_c.dtype)[:, None, None]
    # mask[t, b, l] -> want [t, l, b]
    mask = mask.transpose(0, 2, 1)
    gt = np.where(mask, ft, np.float32(MASK_NEG))
    staged = np.empty((4, T, S, BS), np.float32)
    staged[0] = ft[:, :S, :]
    staged[1] = gt[:, :S, :]
    staged[2] = ft[:, ::-1, :][:, :S, :]
    staged[3] = gt[:, ::-1, :][:, :S, :]
    return staged.reshape(128, S * BS).astype(ml_dtypes.bfloat16)


LAST_RESULTS = None


def kernel(feats, transitions, tags, _trace=False):
    global _compiled, LAST_RESULTS
    from concourse.bass_utils import run_bass_kernel_spmd

    feats = np.asarray(feats, dtype=np.float32)
    transitions = np.asarray(transitions, dtype=np.float32)
    tags = np.asarray(tags)

    if _compiled is None:
        _compiled = _build_nc()
    nc = _compiled

    in_maps = []
    for c in range(NCORES):
        sl = slice(c * BS, (c + 1) * BS)
        in_maps.append(
            {
                "staged": _stage_core(feats[sl], tags[sl]),
                "trans": transitions,
            }
        )
    res = run_bass_kernel_spmd(
        nc, in_maps, core_ids=list(range(NCORES)), trace=_trace
    )
    LAST_RESULTS = res
    out = np.concatenate([r["out"].reshape(BS) for r in res.results])
    return out.astype(np.float32)
